# revision 33
# baseline (speedup 1.0000x reference)
"""3x3 neighborhood (ADDA) attention on Trainium2, B=8, d=512 (8 heads x 64), 56x56.

Sharding: pure data parallel — batch b -> NeuronCore b (8 cores, SPMD, no
cross-core communication). Each core computes full local attention for one
batch.

v8 design (per core, 4 head-pair groups; partitions [0:116) = 2 heads x 58
padded rows, x padded to 60):

  The DMA system here is descriptor-generation bound (~100ns/descriptor,
  one descriptor per partition row per DMA), so the schedule minimizes
  DESCRIPTOR COUNT, not bytes: each tensor arrives as ONE DMA per group
  (k's three host-prepared dy variants share a single 23KB/row DMA), the
  output leaves as one DMA per group (two for the last group to shorten
  the tail), and the partition-shifted AV weights are produced by the
  TensorEngine (matmul with the A_dy shift matrices) + a tiny ACT copy —
  zero descriptors.

  v arrives as TWO x-parity variants (not six dy x parity): the row shift
  for the AV stage is folded into the PE accumulation,
      out[y] = sum_j A_dy(j) @ ( W_j[y-dy] * v[y] ),
  using three shifted-identity matrices A_dy; the same matrices produce
  the shifted weights W_j[y-dy].  Zero-padded image rows keep every ta row
  finite (the PE propagates 0*NaN).

  QK stage: 9 DVE bf16 muls (2x mode) into per-dy-batch [116,3,56,64]
  tiles; the channel reduce is a pairwise tree with all 3 offsets of a
  batch fused per instruction, finishing with a tensor_reduce over the
  last 4.  All elementwise work stays on DVE (GPSIMD compute shares an
  SBUF port with DVE and measured 5x slowdowns on concurrent DVE ops).
  Softmax runs without max subtraction (logits are O(5)), 1/sqrt(64)
  folded into the ACT exp, and the 9-way weight sum is a pairwise tree.
  AV products are channel-outer so the weight broadcasts along the outer
  free dim and mults stay in 2x mode; `ta` is buffered 4-deep so the
  HAM-cold PE drains into the next group's QK phase; finished 512-column
  PSUM chunks are copied out by ACT while the last matmuls still run.
"""
import sys

sys.path.insert(0, "/opt/trn_rl_repo")

from contextlib import ExitStack

import ml_dtypes
import numpy as np

import concourse.bacc as bacc
import concourse.tile as tile
from concourse import mybir
from concourse.bass_utils import run_bass_kernel_spmd

B, D, H, W = 8, 512, 56, 56
NH, HD = 8, 64
SCALE = HD ** (-0.5)
N_CORES = 8
NG = 4          # head-pair groups
P58 = 58        # tile rows per head (1 + 56 + 1)
NP = 116        # compute partitions (2 heads x 58)
XT = 60         # padded x extent (even -> interiors stay 4B-aligned in bf16)
XI = 2          # interior x start
FLAT = HD * W   # 3584
BF16 = mybir.dt.bfloat16
F32 = mybir.dt.float32
BF = ml_dtypes.bfloat16

# j = 3*(dy+1) + (dx+1); dy-major so each batch of 3 shares one k variant.
OFFS = [(dy, dx) for dy in (-1, 0, 1) for dx in (-1, 0, 1)]

_NC_CACHE = {}


def _build_program():
    nc = bacc.Bacc("TRN2", target_bir_lowering=False, debug=False,
                   num_devices=N_CORES)
    q_d = nc.declare_dram_parameter("q", [NG, NP, XT, HD], BF16, isOutput=False)
    k_d = nc.declare_dram_parameter("k", [NG, NP, XT, HD], BF16, isOutput=False)
    v_d = nc.declare_dram_parameter("v", [NG, NP, 2, HD, XT], BF16,
                                    isOutput=False)
    a_d = nc.declare_dram_parameter("amat", [NP, 3, NP], BF16, isOutput=False)
    o_d = nc.declare_dram_parameter("out", [NG, NP, HD, W], BF16, isOutput=True)

    with tile.TileContext(nc) as tc:
        with ExitStack() as ctx:
            one_pool = ctx.enter_context(tc.tile_pool(name="one", bufs=1))
            q_pool = ctx.enter_context(tc.tile_pool(name="q", bufs=2))
            k_pool = ctx.enter_context(tc.tile_pool(name="k", bufs=2))
            kv_pool = ctx.enter_context(tc.tile_pool(name="kv", bufs=1))
            v_pool = ctx.enter_context(tc.tile_pool(name="v", bufs=2))
            tm_pool = ctx.enter_context(tc.tile_pool(name="tm", bufs=1))
            tr_pool = ctx.enter_context(tc.tile_pool(name="tr", bufs=1))
            sm_pool = ctx.enter_context(tc.tile_pool(name="sm", bufs=1))
            wsh_pool = ctx.enter_context(tc.tile_pool(name="wsh", bufs=1))
            ta_pool = ctx.enter_context(tc.tile_pool(name="ta", bufs=4))
            ob_pool = ctx.enter_context(tc.tile_pool(name="ob", bufs=1))
            ps_pool = ctx.enter_context(
                tc.tile_pool(name="ps", bufs=1, space="PSUM"))

            amat = one_pool.tile([NP, 3, NP], BF16)
            nc.sync.dma_start(out=amat[:], in_=a_d[:])

            # persistent on-chip k dy-variants (built on the PE each group)
            km1 = kv_pool.tile([NP, XT, HD], BF16, tag="km1")
            kp1 = kv_pool.tile([NP, XT, HD], BF16, tag="kp1")

            def emit_loads(g):
                """One DMA per tensor per group, each on its own ring; q
                rides the (empirically fastest) SWDGE ring and k is
                half-split for group 0 so the PE k-shift starts early."""
                qt = q_pool.tile([NP, XT, HD], BF16, tag="qt")
                kt = k_pool.tile([NP, XT, HD], BF16, tag="kt")
                va = v_pool.tile([NP, 2, HD, XT], BF16, tag="va")
                if g == 0:
                    # spread the critical first-mul operands (q + k) over
                    # all three rings in ~equal pieces
                    HP = NP // 2
                    nc.gpsimd.dma_start(out=qt[0:HP], in_=q_d[g, 0:HP])
                    nc.scalar.dma_start(out=qt[HP:NP], in_=q_d[g, HP:NP])
                    nc.gpsimd.dma_start(out=kt[0:39], in_=k_d[g, 0:39])
                    nc.scalar.dma_start(out=kt[39:78], in_=k_d[g, 39:78])
                    nc.sync.dma_start(out=kt[78:NP], in_=k_d[g, 78:NP])
                else:
                    nc.gpsimd.dma_start(out=qt[:], in_=q_d[g])
                    nc.scalar.dma_start(out=kt[:], in_=k_d[g])
                nc.sync.dma_start(out=va[:], in_=v_d[g])
                return qt, kt, va

            def emit_kshift(kt):
                """Build km1[p]=kt[p-1], kp1[p]=kt[p+1] on the TensorEngine
                via the A_dy shift matrices (exact: pad rows are zero), in
                512-column PSUM chunks drained by ACT copies.  Zero DMA
                descriptors."""
                ktf = kt[:, :, :].rearrange("p a c -> p (a c)")
                for dst, d in ((km1, 0), (kp1, 2)):
                    df = dst[:, :, :].rearrange("p a c -> p (a c)")
                    for ch in range(8):
                        sl = slice(ch * 480, (ch + 1) * 480)
                        pss = ps_pool.tile([NP, 512], F32, tag="pss")
                        nc.tensor.matmul(pss[:, 0:480], amat[:, d, :],
                                         ktf[:, sl], start=True, stop=True)
                        nc.scalar.copy(df[:, sl], pss[:, 0:480])

            tiles = [emit_loads(0)]
            emit_kshift(tiles[0][1])

            for g in range(NG):
                # next group's loads go on the rings BEFORE this group's
                # store so stores never delay loads.
                if g + 1 < NG:
                    tiles.append(emit_loads(g + 1))
                qt, kt, va = tiles[g]

                L = sm_pool.tile([NP, 9, W], F32, tag="L")
                Pt = sm_pool.tile([NP, 9, W], BF16, tag="P")
                Wt = sm_pool.tile([NP, 9, W], BF16, tag="W")
                S = sm_pool.tile([NP, W], F32, tag="S")
                R = sm_pool.tile([NP, W], F32, tag="R")

                # --- QK: logits; batched pairwise channel-reduce tree.
                # dy=0 batch first (k variant 1). ---
                for b in (1, 0, 2):
                    ksrc = {1: kt, 0: km1, 2: kp1}[b]
                    tm = tm_pool.tile([NP, 3, W, HD], BF16, tag="tm")
                    for ji in range(3):
                        dy, dx = OFFS[3 * b + ji]
                        nc.vector.tensor_mul(
                            tm[:, ji, :, :],
                            qt[:, XI:XI + W, :],
                            ksrc[:, XI + dx:XI + dx + W, :],
                        )
                    t32 = tr_pool.tile([NP, 3, W, 32], BF16, tag="t32")
                    nc.vector.tensor_add(t32[:], tm[:, :, :, 0:32],
                                         tm[:, :, :, 32:64])
                    t16 = tr_pool.tile([NP, 3, W, 16], BF16, tag="t16")
                    nc.vector.tensor_add(t16[:], t32[:, :, :, 0:16],
                                         t32[:, :, :, 16:32])
                    t8 = tr_pool.tile([NP, 3, W, 8], BF16, tag="t8")
                    nc.vector.tensor_add(t8[:], t16[:, :, :, 0:8],
                                         t16[:, :, :, 8:16])
                    t4 = tr_pool.tile([NP, 3, W, 4], BF16, tag="t4")
                    nc.vector.tensor_add(t4[:], t8[:, :, :, 0:4],
                                         t8[:, :, :, 4:8])
                    nc.vector.tensor_reduce(
                        out=L[:, 3 * b:3 * b + 3, :], in_=t4[:, :, :, :],
                        axis=mybir.AxisListType.X, op=mybir.AluOpType.add,
                    )

                # --- softmax (no max subtraction; SCALE folded into exp);
                # the 9-way weight sum is a pairwise tree (2x mode). ---
                nc.scalar.activation(
                    out=Pt[:, :, :], in_=L[:, :, :],
                    func=mybir.ActivationFunctionType.Exp, scale=float(SCALE),
                )
                sa = tr_pool.tile([NP, 4, W], BF16, tag="sa")
                nc.vector.tensor_add(sa[:], Pt[:, 0:4, :], Pt[:, 4:8, :])
                sb = tr_pool.tile([NP, 2, W], BF16, tag="sb")
                nc.vector.tensor_add(sb[:], sa[:, 0:2, :], sa[:, 2:4, :])
                sc = tr_pool.tile([NP, W], F32, tag="sc")
                nc.vector.tensor_add(sc[:], sb[:, 0, :], sb[:, 1, :])
                nc.vector.tensor_add(S[:], sc[:], Pt[:, 8, :])
                nc.vector.reciprocal(out=R[:, :], in_=S[:, :])
                nc.vector.tensor_mul(
                    Wt[:, :, :],
                    Pt[:, :, :],
                    R[:, :].unsqueeze(1).to_broadcast((NP, 9, W)),
                )

                # shifted weights for the dy=+-1 AV batches, produced on the
                # TensorEngine with the same A_dy shift matrices (zero DMA
                # descriptors): wm1[p] = W[p+1], wp1[p] = W[p-1].
                wshp = ps_pool.tile([NP, 512], F32, tag="pss")
                nc.tensor.matmul(wshp[:, 0:168], amat[:, 2, :],
                                 Wt[:, 0:3, :], start=True, stop=True)
                nc.tensor.matmul(wshp[:, 168:336], amat[:, 0, :],
                                 Wt[:, 6:9, :], start=True, stop=True)
                wm1 = wsh_pool.tile([NP, 3, W], BF16, tag="wm1")
                wp1 = wsh_pool.tile([NP, 3, W], BF16, tag="wp1")
                nc.scalar.copy(wm1[:], wshp[:, 0:168].rearrange(
                    "p (a b) -> p a b", a=3))
                nc.scalar.copy(wp1[:], wshp[:, 168:336].rearrange(
                    "p (a b) -> p a b", a=3))

                # --- AV: dy=0 first (overlaps the weight shift); PE
                # accumulates with the A_dy shift matrices. ---
                av = ps_pool.tile([NP, FLAT], F32, tag="av")
                ob = ob_pool.tile([NP, FLAT], BF16, tag="ob")
                od_flat = o_d[g].rearrange("p c x -> p (c x)")
                pos = 0
                for b in (1, 0, 2):
                    for ji in range(3):
                        j = 3 * b + ji
                        dy, dx = OFFS[j]
                        xp = dx & 1
                        xb = XI + xp + dx
                        if dy == 0:
                            w_ap = Wt[:, j:j + 1, :]
                        elif dy == -1:
                            w_ap = wm1[:, ji:ji + 1, :]
                        else:
                            w_ap = wp1[:, ji:ji + 1, :]
                        ta = ta_pool.tile([NP, HD, W], BF16, tag="ta")
                        nc.vector.tensor_mul(
                            ta[:, :, :],
                            w_ap.to_broadcast((NP, HD, W)),
                            va[:, xp, :, xb:xb + W],
                        )
                        taf = ta[:, :, :].rearrange("p c x -> p (c x)")
                        for ch in range(FLAT // 512):
                            nc.tensor.matmul(
                                av[:, ch * 512:(ch + 1) * 512],
                                amat[:, b, :],
                                taf[:, ch * 512:(ch + 1) * 512],
                                start=(pos == 0),
                                stop=(pos == 8),
                            )
                        pos += 1
                # two half-copies (whole-tile WAR tracking would make
                # finer chunks ping-pong with the last j's matmuls)
                HF = FLAT // 2
                nc.scalar.copy(ob[:, 0:HF], av[:, 0:HF])
                nc.scalar.copy(ob[:, HF:FLAT], av[:, HF:FLAT])
                if g + 1 < NG:
                    nc.gpsimd.dma_start(out=od_flat[:], in_=ob[:])
                    # build the NEXT group's k dy-variants on the PE now
                    # (after this group's AV matmuls in the PE queue).
                    emit_kshift(tiles[g + 1][1])
                else:
                    # split the last store across the two HWDGE rings in
                    # near-equal pieces (SWDGE's end-drain is ~10us)
                    nc.sync.dma_start(out=od_flat[:, 0:1152],
                                      in_=ob[:, 0:1152])
                    nc.scalar.dma_start(out=od_flat[:, 1152:2304],
                                        in_=ob[:, 1152:2304])
                    nc.sync.dma_start(out=od_flat[:, 2304:FLAT],
                                      in_=ob[:, 2304:FLAT])

    nc.compile()
    return nc


def _get_nc():
    if "nc" not in _NC_CACHE:
        _NC_CACHE["nc"] = _build_program()
    return _NC_CACHE["nc"]


def _prep_inputs(q, k, v):
    """Build per-core images (leading dim = core/batch).

    q: [B, NG, 116, 60, 64]; k: [B, NG, 116, 3, 60, 64] (dy in {-1,0,1});
    v: [B, NG, 116, 2, 64, 60] (x-parity variants).
    Tile row p = hh*58 + pr holds image row y = pr - 1 (+dy for k variants);
    out-of-range rows and x pads are zero.  amat[p, d, y] = 1 iff ta-row p
    feeds out-row y for dy = d-1 (y = p - dy), edge rows routed to their own
    (pad) row.
    """
    qyxc = q.reshape(B, NH, HD, H, W).transpose(0, 1, 3, 4, 2).astype(BF)
    kyxc = k.reshape(B, NH, HD, H, W).transpose(0, 1, 3, 4, 2).astype(BF)
    vycx = v.reshape(B, NH, HD, H, W).transpose(0, 1, 3, 2, 4).astype(BF)

    qi = np.zeros((B, NG, NP, XT, HD), dtype=BF)
    ki = np.zeros((B, NG, NP, XT, HD), dtype=BF)
    vi = np.zeros((B, NG, NP, 2, HD, XT), dtype=BF)
    for g in range(NG):
        for hh in range(2):
            hd = 2 * g + hh
            p0 = hh * P58
            qi[:, g, p0 + 1:p0 + 1 + H, XI:XI + W, :] = qyxc[:, hd]
            ki[:, g, p0 + 1:p0 + 1 + H, XI:XI + W, :] = kyxc[:, hd]
            for xp in (0, 1):
                vi[:, g, p0 + 1:p0 + 1 + H, xp, :, XI + xp:XI + xp + W] = \
                    vycx[:, hd]
    amat = np.zeros((NP, 3, NP), dtype=BF)
    for d, dy in enumerate((-1, 0, 1)):
        for p in range(NP):
            y = p - dy
            amat[p, d, y if 0 <= y < NP else p] = 1
    return [{"q": qi[b], "k": ki[b], "v": vi[b], "amat": amat}
            for b in range(N_CORES)]


def _run(q, k, v, trace=False, tmpdir=None):
    q = np.asarray(q, dtype=np.float32)
    k = np.asarray(k, dtype=np.float32)
    v = np.asarray(v, dtype=np.float32)
    in_maps = _prep_inputs(q, k, v)
    nc = _get_nc()
    res = run_bass_kernel_spmd(nc, in_maps, core_ids=list(range(N_CORES)),
                               trace=trace, tmpdir=tmpdir)
    # out image [NG, 116, 64, 56] -> [y, x, c]
    out = np.empty((B, H, W, D), dtype=np.float32)
    for b in range(N_CORES):
        oi = np.asarray(res.results[b]["out"]).astype(np.float32)
        for g in range(NG):
            for hh in range(2):
                hd = 2 * g + hh
                blk = oi[g, hh * P58 + 1:hh * P58 + 1 + H]     # [y, c, x]
                out[b, :, :, hd * HD:(hd + 1) * HD] = blk.transpose(0, 2, 1)
    return out, res


def kernel(q, k, v):
    out, _ = _run(q, k, v, trace=False)
    return out


def run_traced(q, k, v, tmpdir=None):
    out, res = _run(q, k, v, trace=True, tmpdir=tmpdir)
    return out, res
